# revision 30
# baseline (speedup 1.0000x reference)
"""AxialAttention (MSA row attention) Trainium2 Bass kernel, 8-core SPMD.

Sharding: the s=128 MSA-row axis is split 16 rows/core across 8 cores.
Params are replicated; the pairwise attention bias is recomputed on every
core from a CPU-pre-transposed fp8 copy of `edges`.

v2 design (fp8 DoubleRow everywhere K>=128 allows):
  LayerNorm (tokens on partitions, bn_stats; rstd = exp(-0.5 ln(var+eps)))
  PE-transpose xc -> xcT fp8 [d, 2, tok]
  q/k/g/v projections: fp8 DoubleRow matmuls (weights pre-scaled x64/x256
  on CPU to dodge fp8 subnormals; compensated at psum readout)
  scoresT[j,i] = kT.T@qT per head (bf16, no bias inject)
  P = exp(scoresT) * expB  (expB = exp(bias) precomputed once; multiply on
  GpSimd)  -> fp8
  av + Z in one pass: av = v.T@P (DR); Z broadcast over 64 partitions via
  ones-lhsT DR matmul (row-sum + partition broadcast in one instruction)
  gate: sigmoid via exp table: t1=(1+e^-z)*Z; gatedT = av / t1 (DVE stt)
  out = gatedT.T @ Wo (DR) ; fout = psum*2^-6 + bo_bcast -> DRAM
The bias phase (Web.T @ edgesT, fp8 DR) is interleaved with rows 0-3,
stored padded to DRAM, loaded back and exp'd into SBUF as expB.
"""
import sys

if "/opt/trn_rl_repo" not in sys.path:
    sys.path.insert(0, "/opt/trn_rl_repo")

import numpy as np
import ml_dtypes

import concourse.bass as bass
import concourse.tile as tile
from concourse import bacc, mybir
from concourse.bass_utils import run_bass_kernel_spmd

F32 = mybir.dt.float32
BF16 = mybir.dt.bfloat16
FP8 = mybir.dt.float8e4
AF = mybir.ActivationFunctionType
ALU = mybir.AluOpType
DR = mybir.MatmulPerfMode.DoubleRow

N_CORES = 8
S = 128                 # MSA rows (axial batch)
S_PER_CORE = S // N_CORES
N = 256                 # sequence positions per row
D = 256                 # node dim
HEADS = 8
DH = 64                 # head dim
DI = HEADS * DH         # 512
DE = 128                # edge dim
T_EDGE = N * N          # 65536 flattened (j,i) pairs
EDGE_CHUNK = 4096       # pairs per bias-phase chunk (fp8: 512 KB)
N_CHUNKS = T_EDGE // EDGE_CHUNK
SCALE = DH ** -0.5
SQ = 2.0 ** -8          # wq pre-scale compensation
SW = 2.0 ** -6          # wk/wv/wg/wo/web pre-scale compensation
SL = 5                  # scores lag projmm by this many iterations
BIAS_ROWS = 4           # bias chunks spread over this many leading rows


def build_nc(has_beta, gb_const):
    nc = bacc.Bacc("TRN2", target_bir_lowering=False, debug=False,
                   num_devices=N_CORES)

    io = {}
    io["x"] = nc.dram_tensor("x", [S_PER_CORE * N, D], BF16, kind="ExternalInput").ap()
    io["edgesT"] = nc.dram_tensor("edgesT", [128, T_EDGE], FP8, kind="ExternalInput").ap()
    io["wq"] = nc.dram_tensor("wq", [128, 2, DI], FP8, kind="ExternalInput").ap()
    io["wk"] = nc.dram_tensor("wk", [128, 2, DI], FP8, kind="ExternalInput").ap()
    io["wg"] = nc.dram_tensor("wg", [128, 2, DI], FP8, kind="ExternalInput").ap()
    io["wv"] = nc.dram_tensor("wv", [128, 2, DI], BF16, kind="ExternalInput").ap()
    io["wo"] = nc.dram_tensor("wo", [128, 4, D], BF16, kind="ExternalInput").ap()
    io["web"] = nc.dram_tensor("web", [128, 64], FP8, kind="ExternalInput").ap()
    io["bcols"] = nc.dram_tensor("bcols", [128, 12], F32, kind="ExternalInput").ap()
    io["bwv_b"] = nc.dram_tensor("bwv_b", [128, DI], F32, kind="ExternalInput").ap()
    io["bo_b"] = nc.dram_tensor("bo_b", [128, D], F32, kind="ExternalInput").ap()
    io["consts"] = nc.dram_tensor("consts", [128, 256], FP8, kind="ExternalInput").ap()
    io["eps"] = nc.dram_tensor("eps", [128, 2], F32, kind="ExternalInput").ap()
    io["out"] = nc.dram_tensor("out", [S_PER_CORE * N, D], F32, kind="ExternalOutput").ap()

    with tile.TileContext(nc) as tc, nc.allow_low_precision(
        reason="fp8/bf16 matmul operands; fp32 PSUM accumulation"
    ):
        _emit(nc, tc, io, has_beta, gb_const)
    nc.compile()
    return nc


CAT_MAP = {}


def _emit(nc, tc, io, has_beta, gb_const):
    from contextlib import ExitStack
    ctx = ExitStack()
    const = ctx.enter_context(tc.tile_pool(name="const", bufs=1))
    work = ctx.enter_context(tc.tile_pool(name="work", bufs=2))
    row = ctx.enter_context(tc.tile_pool(name="row", bufs=SL + 2))
    small = ctx.enter_context(tc.tile_pool(name="small", bufs=4))
    edg = ctx.enter_context(tc.tile_pool(name="edg", bufs=4))
    ps_s = ctx.enter_context(tc.tile_pool(name="ps_s", bufs=3, space="PSUM"))
    ps_az = ctx.enter_context(tc.tile_pool(name="ps_az", bufs=2, space="PSUM"))
    ps_p = ctx.enter_context(tc.tile_pool(name="ps_p", bufs=3, space="PSUM"))
    dram = ctx.enter_context(tc.tile_pool(name="dram", bufs=1, space="DRAM"))

    def mm(cat, *a, **kw):
        r = nc.tensor.matmul(*a, **kw)
        try:
            CAT_MAP[r.ins.name] = cat
        except AttributeError:
            pass
        return r

    def tp(cat, *a, **kw):
        r = nc.tensor.transpose(*a, **kw)
        try:
            CAT_MAP[r.ins.name] = cat
        except AttributeError:
            pass
        return r

    # ---- constants / weights ----
    consts_sb = const.tile([128, 256], FP8)
    nc.sync.dma_start(consts_sb, io["consts"])
    ident_f8 = consts_sb[:, 0:128]

    wq_sb = const.tile([128, 2, DI], FP8)
    nc.sync.dma_start(wq_sb, io["wq"])
    wk_sb = const.tile([128, 2, DI], FP8)
    nc.sync.dma_start(wk_sb, io["wk"])
    wg_sb = const.tile([128, 2, DI], FP8)
    nc.sync.dma_start(wg_sb, io["wg"])
    wv_sb = const.tile([128, 2, DI], BF16)
    nc.sync.dma_start(wv_sb, io["wv"])
    wo_sb = const.tile([128, 4, D], BF16)
    nc.sync.dma_start(wo_sb, io["wo"])
    web_sb = const.tile([128, 64], FP8)
    nc.sync.dma_start(web_sb, io["web"])
    bcols = const.tile([128, 12], F32)   # bwq | bwk | gb cols (4 each)
    nc.sync.dma_start(bcols, io["bcols"])
    bwq_col = bcols[:, 0:4]
    bwk_col = bcols[:, 4:8]
    gb_col = bcols[:, 8:12]
    bwv_b = const.tile([128, DI], F32)
    nc.sync.dma_start(bwv_b, io["bwv_b"])
    bo_b = const.tile([128, D], F32)
    nc.sync.dma_start(bo_b, io["bo_b"])
    eps2 = const.tile([128, 2], F32)
    nc.sync.dma_start(eps2, io["eps"])
    eps_col = eps2[:, 0:1]
    gbc_col = eps2[:, 1:2]

    x_all = const.tile([128, S_PER_CORE, 2, 256], BF16)
    mv_all = const.tile([128, S_PER_CORE, 2, 2], F32)
    rstd_all = const.tile([128, S_PER_CORE, 2, 1], F32)

    biasPad = dram.tile([HEADS, 8 * N_CHUNKS * 2, 2, 256], BF16)
    biasT_sb = const.tile([128, 2 * HEADS, 256], BF16)   # [j, (h,jt), i]


    # ---- bias phase pieces ----
    DMAQ = None

    def emit_bias_chunk(c):
        e_dr = edg.tile([128, EDGE_CHUNK], FP8, tag="edg", name="e_dr")
        (nc.sync, nc.scalar)[c % 2].dma_start(
            e_dr, io["edgesT"][:, c * EDGE_CHUNK:(c + 1) * EDGE_CHUNK])
        for b in range(4):          # banklet: 1024 pairs, 2 groups at parts 0/64
            pb = ps_s.tile([128, 512], F32, tag="ps", name="pb")
            for g in range(2):
                off = b * 1024 + g * 512
                mm("bias", pb[64 * g:64 * g + 64, :], web_sb,
                   e_dr[:, off:off + 512],
                   start=True, stop=True, skip_group_check=True)
            pb_sb = edg.tile([128, 512], BF16, tag="pb_sb", name="pb_sb")
            if b % 2 == 0:
                nc.vector.tensor_scalar_mul(pb_sb, pb, SW)
            else:
                nc.scalar.mul(pb_sb, pb, SW)
            for g in range(2):
                sb = c * 8 + b * 2 + g
                q = nc.gpsimd
                q.dma_start(biasPad[:, sb],
                            pb_sb[64 * g:64 * g + HEADS]
                            .rearrange("h (j0 i) -> h j0 i", j0=2))

    def emit_bias_loadback(jt):
        for h in range(HEADS):
            src = biasPad[h, jt * 64:(jt + 1) * 64, :, :]
            nc.sync.dma_start(
                biasT_sb[:, h * 2 + jt],
                src.rearrange("sb j0 i -> (sb j0) i"))


    # ---- per-row stages ----
    prep_tiles = {}   # r -> xcT
    proj_tiles = {}   # r -> (qT, kT, gTp, v_sb)
    pT_tiles = {}     # r -> list of (pair, [pT_idx0, pT_idx1])

    def emit_x_group(g):
        # DMA 4 rows of x and compute LN stats + rstd for them, batched so
        # the Ln/Exp act-table dance happens once per group, not per row.
        r0 = g * 4
        nc.gpsimd.dma_start(
            x_all[:, r0:r0 + 4],
            io["x"][r0 * N:(r0 + 4) * N].rearrange("(r t p) d -> p r t d",
                                                   p=128, t=2))
        st = small.tile([128, 6], F32, tag="st", name="st")
        for r in range(r0, r0 + 4):
            for tt in range(2):
                nc.vector.bn_stats(st, x_all[:, r, tt])
                nc.vector.bn_aggr(mv_all[:, r, tt], st)
        lnv = small.tile([128, 4, 2, 1], F32, tag="lnv", name="lnv")
        nc.scalar.activation(lnv, mv_all[:, r0:r0 + 4, :, 1:2], AF.Ln,
                             bias=eps_col)
        nc.scalar.activation(rstd_all[:, r0:r0 + 4], lnv, AF.Exp, scale=-0.5)

    def emit_prep(r):
        nmr = small.tile([128, 2], F32, tag="nmr", name="nmr")
        xc = work.tile([128, 2, D], BF16, tag="xc", bufs=2, name="xc")
        for tt in range(2):
            nc.vector.tensor_scalar(nmr[:, tt:tt + 1], mv_all[:, r, tt, 0:1],
                                    rstd_all[:, r, tt], -1.0, ALU.mult, ALU.mult)
            nc.vector.tensor_scalar(xc[:, tt], x_all[:, r, tt],
                                    rstd_all[:, r, tt], nmr[:, tt:tt + 1],
                                    ALU.mult, ALU.add)
        # transpose -> [d, dt, tok] (emitted late in PE stream by caller)
        return xc

    # transpose needs a bf16 identity; build one by casting once.
    ident_bf = const.tile([128, 128], BF16)
    nc.vector.tensor_copy(ident_bf, ident_f8)
    ones_bf = const.tile([128, 64], BF16)
    nc.vector.memset(ones_bf, 1.0)


    def emit_prep_tp2(r, xc):
        pxt = ps_p.tile([128, 2, 256], BF16, tag="ps", name="pxt")
        for dt in range(2):
            for tt in range(2):
                tp("xcT", pxt[:, dt, tt * 128:(tt + 1) * 128],
                   xc[:, tt, dt * 128:(dt + 1) * 128], ident_bf)
        xcT = work.tile([128, 2, N], FP8, tag="xcT", bufs=3, name="xcT")
        nc.vector.tensor_copy(xcT, pxt)
        xcTb = work.tile([128, 2, N], BF16, tag="xcTb", bufs=3, name="xcTb")
        nc.vector.tensor_copy(xcTb, pxt)
        prep_tiles[r] = (xcT, xcTb)

    def emit_projmm(r):
        xcT, xcTb = prep_tiles.pop(r)
        qT = row.tile([128, 4, N], BF16, tag="qT", name="qT")
        kT = row.tile([128, 4, N], BF16, tag="kT", name="kT")
        gTp = row.tile([128, 4, N], BF16, tag="gTp", name="gTp")
        v_sb = row.tile([128, 2, DI], BF16, tag="v", name="v_sb")

        for w_sb, dst, bcol, comp, kind in (
                (wq_sb, qT, bwq_col, SQ, "q"),
                (wk_sb, kT, bwk_col, SW, "k"),
                (wg_sb, gTp, gb_col, SW, "g")):
            for fp in range(2):
                p = ps_p.tile([128, 2, 256], F32, tag="ps", name="p_proj")
                for sub in range(2):
                    fb = fp * 2 + sub
                    mm("proj", p[:, sub], w_sb[:, :, fb * 128:(fb + 1) * 128],
                       xcT, perf_mode=DR, start=True, stop=True,
                       skip_group_check=True)
                if kind == "g":
                    # gTp = exp(-(psum*SW + gb)): sigmoid via exp table
                    if gb_const is not None:
                        nc.scalar.activation(gTp[:, fp * 2:fp * 2 + 2], p,
                                             AF.Exp, scale=-SW, bias=gbc_col)
                    else:
                        for sub in range(2):
                            fb = fp * 2 + sub
                            nc.scalar.activation(gTp[:, fb], p[:, sub], AF.Exp,
                                                 scale=-SW,
                                                 bias=gb_col[:, fb:fb + 1])
                elif not has_beta:
                    if kind == "k":
                        nc.scalar.mul(dst[:, fp * 2:fp * 2 + 2], p, comp)
                    else:
                        nc.vector.tensor_scalar_mul(dst[:, fp * 2:fp * 2 + 2], p, comp)
                else:
                    for sub in range(2):
                        fb = fp * 2 + sub
                        nc.vector.tensor_scalar(dst[:, fb], p[:, sub],
                                                comp, bcol[:, fb:fb + 1],
                                                ALU.mult, ALU.add)

        for tt in range(2):
            pv = ps_p.tile([128, 512], F32, tag="ps", name="pv")
            for kt in range(2):
                mm("vproj", pv, xcTb[:, kt, tt * 128:(tt + 1) * 128],
                   wv_sb[:, kt], start=(kt == 0), stop=(kt == 1))
            if not has_beta:
                nc.vector.tensor_copy(v_sb[:, tt], pv)
            else:
                nc.vector.scalar_tensor_tensor(v_sb[:, tt], pv, 1.0, bwv_b,
                                               ALU.mult, ALU.add)
        proj_tiles[r] = (qT, kT, gTp, v_sb)

    def emit_scores_pair(r, pair):
        qT, kT, gTp, v_sb = proj_tiles[r]
        ft = pair
        pTs = []
        for idx in range(2):
            h = 2 * pair + idx
            ph = idx * 64
            s_ps = ps_s.tile([128, 512], F32, tag="ps", name="s_ps")
            mm("inject", s_ps, ident_bf, biasT_sb[:, h * 2:h * 2 + 2],
               start=True, stop=True)
            for jt in range(2):
                mm("qk", s_ps[:, jt * 256:(jt + 1) * 256],
                   kT[ph:ph + 64, ft, jt * 128:(jt + 1) * 128],
                   qT[ph:ph + 64, ft],
                   start=False, stop=True, skip_group_check=True)
            pT = work.tile([128, 2, 256], BF16, tag="pT", bufs=12, name="pT")
            nc.scalar.activation(pT, s_ps.rearrange("p (t i) -> p t i", t=2),
                                 AF.Exp)
            pTs.append(pT)
        pT_tiles.setdefault(r, []).append(pTs)

    def emit_avout_pair(r, pair, gatedT):
        _, _, gTp, v_sb = proj_tiles[r]
        pTs = pT_tiles[r][pair]
        az = ps_az.tile([128, 2, 256], F32, tag="ps", name="az")
        for idx in range(2):
            h = 2 * pair + idx
            ph = idx * 64
            for jt in range(2):
                mm("av", az[ph:ph + 64, 0, :],
                   v_sb[:, jt, h * DH:(h + 1) * DH], pTs[idx][:, jt, :],
                   start=(jt == 0), stop=(jt == 1), skip_group_check=True)
            for jt in range(2):
                mm("zs", az[ph:ph + 64, 1, :],
                   ones_bf, pTs[idx][:, jt, :],
                   start=(jt == 0), stop=(jt == 1), skip_group_check=True)
        t1 = small.tile([128, 256], F32, tag="t1", name="t1")
        nc.vector.scalar_tensor_tensor(t1, gTp[:, pair], 1.0,
                                       az[:, 1, :], ALU.add, ALU.mult)
        rt1 = small.tile([128, 256], F32, tag="rt1", name="rt1")
        nc.vector.reciprocal_approx_fast(rt1, t1)
        nc.vector.tensor_tensor(gatedT[:, pair], az[:, 0, :], rt1, ALU.mult)

    def emit_final(r, gatedT):
        pf = ps_az.tile([128, 2, 256], F32, tag="ps", name="pf")
        for tt in range(2):
            for kt in range(4):
                mm("final", pf[:, tt],
                   gatedT[:, kt, tt * 128:(tt + 1) * 128],
                   wo_sb[:, kt, :],
                   start=(kt == 0), stop=(kt == 3),
                   skip_group_check=True)
        fout = work.tile([128, 2, 256], F32, tag="fout", bufs=3, name="fout")
        for tt in range(2):
            nc.vector.tensor_tensor(fout[:, tt], pf[:, tt], bo_b, ALU.add)
        nc.gpsimd.dma_start(io["out"][r * N:(r + 1) * N]
                          .rearrange("(t p) d -> p t d", p=128), fout)
        del pT_tiles[r]

    # ---- interleaved pipeline ----
    # iteration it: projmm(it) (xcT prepared last iteration), scores(it-SL)
    # interleaved with avout(it-SL-1); xc+transposes for row it+1 at the end.
    TOTAL = S_PER_CORE + SL + 1
    for it in range(TOTAL):
        r_proj = it
        r_sc = it - SL
        r_av = it - 1 - SL

        if it == 0:
            emit_x_group(0)
            emit_prep_tp2(0, emit_prep(0))
        if it < BIAS_ROWS:
            for c in range(4 * it, 4 * it + 4):
                emit_bias_chunk(c)
        if it == 2:
            emit_bias_loadback(0)
        if it == 4:
            emit_bias_loadback(1)
        if r_proj < S_PER_CORE:
            emit_projmm(r_proj)

        gatedT = None
        if 0 <= r_av < S_PER_CORE:
            gatedT = work.tile([128, 4, N], BF16, tag="gatedT", bufs=2,
                               name="gatedT")
        # interleave scores pairs with avout pairs
        for p in range(4):
            if 0 <= r_sc < S_PER_CORE:
                emit_scores_pair(r_sc, p)
            if gatedT is not None:
                emit_avout_pair(r_av, p, gatedT)
        if gatedT is not None:
            emit_final(r_av, gatedT)
        # release proj tiles consumed by the avout row
        if 0 <= r_av < S_PER_CORE:
            proj_tiles.pop(r_av, None)
        # prepare next row: xc + transposes + xcT copies
        if it + 1 < S_PER_CORE:
            emit_prep_tp2(it + 1, emit_prep(it + 1))
        # prefetch next 4-row x group + its LN stats at end of iteration
        if 1 <= it + 1 <= 3:
            emit_x_group(it + 1)

    ctx.close()


_NC_CACHE = {}


def _get_nc(has_beta, gb_const):
    key = (has_beta, gb_const)
    if key not in _NC_CACHE:
        _NC_CACHE[key] = build_nc(has_beta, gb_const)
    return _NC_CACHE[key]


def make_in_maps(x, edges, mask, gamma, beta, Wq, Wkv, Wo, bo, Wg, bg, Web):
    f32 = np.float32
    bf16 = ml_dtypes.bfloat16
    fp8 = ml_dtypes.float8_e4m3

    gamma = np.asarray(gamma, f32)
    beta = np.asarray(beta, f32)
    Wq = np.asarray(Wq, f32)
    Wkv = np.asarray(Wkv, f32)
    Wk = Wkv[:, :DI]
    Wv = Wkv[:, DI:]
    Wg = np.asarray(Wg, f32)
    Wo_ = np.asarray(Wo, f32)
    bg = np.asarray(bg, f32).reshape(-1)
    bo = np.asarray(bo, f32).reshape(-1)
    Web_ = np.asarray(Web, f32)

    def dr2(w, sc):   # [256, F] -> [128, 2, F]
        return np.ascontiguousarray(
            (w * sc).reshape(2, 128, -1).transpose(1, 0, 2)).astype(fp8)

    wq_dr = dr2(Wq * gamma[:, None], SCALE * 256.0)
    wk_dr = dr2(Wk * gamma[:, None], 64.0)
    wg_dr = dr2(Wg * gamma[:, None], 64.0)
    wv_dr = np.ascontiguousarray(
        (Wv * gamma[:, None]).reshape(2, 128, DI).transpose(1, 0, 2)).astype(bf16)
    wo_dr = np.ascontiguousarray(
        Wo_.reshape(4, 128, D).transpose(1, 0, 2)).astype(bf16)

    bwq = (beta @ Wq) * SCALE
    bwk = beta @ Wk
    bwv = beta @ Wv
    gb = -(beta @ Wg + bg)
    bcols = np.stack([bwq.reshape(4, 128), bwk.reshape(4, 128),
                      gb.reshape(4, 128)], axis=0)  # [3, 4, 128]
    bcols = np.ascontiguousarray(bcols.reshape(12, 128).T).astype(f32)  # [128,12]

    has_beta = bool(np.any(beta != 0.0))
    gb_const = None
    if not has_beta and np.all(bg == bg.flat[0]):
        gb_const = float(-bg.flat[0])

    E = np.asarray(edges, f32)[0].transpose(1, 0, 2).reshape(T_EDGE, DE).T
    edgesT_dr = np.ascontiguousarray(E).astype(fp8)
    web_dr = np.ascontiguousarray(np.concatenate(
        [Web_ * 64.0, np.zeros((DE, 56), f32)], axis=1)).astype(fp8)

    consts = np.concatenate([np.eye(128, dtype=f32),
                             np.ones((128, 128), f32)], axis=1).astype(fp8)

    shared = {
        "edgesT": edgesT_dr,
        "wq": wq_dr, "wk": wk_dr, "wg": wg_dr, "wv": wv_dr, "wo": wo_dr,
        "web": web_dr,
        "bcols": bcols,
        "bwv_b": np.ascontiguousarray(np.tile(bwv[None, :], (128, 1))).astype(f32),
        "bo_b": np.ascontiguousarray(np.tile(bo[None, :], (128, 1))).astype(f32),
        "consts": consts,
        "eps": np.stack([np.full(128, 1e-5, f32),
                         np.full(128, gb_const if gb_const is not None else 0.0,
                                 f32)], axis=1),
    }
    x0 = np.asarray(x, f32)[0]   # [S, N, D]
    in_maps = []
    for c in range(N_CORES):
        xs = np.ascontiguousarray(
            x0[c * S_PER_CORE:(c + 1) * S_PER_CORE].reshape(S_PER_CORE * N, D)
        ).astype(bf16)
        in_maps.append({"x": xs, **shared})
    return in_maps, has_beta, gb_const


def kernel(x, edges, mask, gamma, beta, Wq, Wkv, Wo, bo, Wg, bg, Web,
           **run_kwargs):
    in_maps, has_beta, gb_const = make_in_maps(
        x, edges, mask, gamma, beta, Wq, Wkv, Wo, bo, Wg, bg, Web)
    nc = _get_nc(has_beta, gb_const)
    res = run_bass_kernel_spmd(nc, in_maps, core_ids=list(range(N_CORES)),
                               **run_kwargs)
    outs = [res.results[c]["out"].reshape(S_PER_CORE, N, D) for c in range(N_CORES)]
    full = np.concatenate(outs, axis=0)[None]   # [1, S, N, D]
    if run_kwargs:
        kernel.last_results = res
    return full
